# revision 24
# baseline (speedup 1.0000x reference)
"""EmbeddingBag-mean (padded ragged gather + masked mean) on 8 Trainium2 cores.

Strategy (data-parallel over batch):
  - Each of the 8 cores owns B/8 = 2048 samples; the embedding table is
    replicated to every core's HBM as fp16, rows padded to 128 elements
    (256 B stride) so the MoE `dma_gather` ucode (stride in 256 B units,
    int16 indices) can gather single 128 B rows.
  - int16 indices only reach 32768 rows, so the gather runs as 4 passes
    over overlapping 32768-row windows (bases ~22412 apart).  A zero
    sentinel row sits at each window base (relative index 0).  Because the
    windows overlap, each sample distributes its indices among feasible
    passes to equalize its per-pass counts, which keeps the per-block
    per-pass slot maxima near len/4 (instead of len/4 + 3 sigma).
  - Samples are globally length-sorted and dealt to (block, core,
    partition) so each block of 128 partitions holds samples of nearly
    equal length; block b needs G[b][q] gather slots for pass q (max over
    the 8 cores' blocks; one SPMD module).  Pad slots -> sentinel row.
  - Device kernel (per core), per block of 128 samples:
      1. 4x dma_gather (one per pass, 4 SWDGE queues round-robin):
         g[p, off_q + j, :] = window_q[idx16[...], :64]   (128 B descs)
      2. one DVE tensor_reduce over all slot columns (fp16 in, fp32 out)
      3. ACT Copy-with-scale by 1/max(len,1) (per-partition scalar)
      4. DMA the [128, 64] fp32 block out
  - Host un-permutes the global deal and returns [B, 64] fp32.
"""

import numpy as np

try:
    import concourse.bacc as bacc
except ImportError:  # harness containers keep the repo at /opt/trn_rl_repo
    import sys

    sys.path.insert(0, "/opt/trn_rl_repo")
    import concourse.bacc as bacc

import concourse.bass as bass
import concourse.mybir as mybir
import concourse.tile as tile
from concourse import bass_utils

B, L, V, D = 16384, 50, 100000, 64
NCORES = 8
P = 128
BC = B // NCORES  # 2048 samples per core
NBLK = BC // P  # 16 blocks of 128 samples
NQ = 4  # gather passes (overlapping windows)
WIN = 32768  # int16-reachable rows per pass
DEVROWS = V + NQ  # table + one zero sentinel per window

# window bases in device-table row space (sentinel zero row at each base)
_SPACING = -(-(DEVROWS - WIN) // (NQ - 1))  # ceil
BASES = [q * _SPACING for q in range(NQ)]
assert BASES[-1] + WIN >= DEVROWS

_CACHE: dict = {}
TRIM = False


def _manual_dma_gather(nc, out_ap, in_ap, idxs_ap, num_idxs, elem_size,
                       queue_num, single_packet):
    """dma_gather without the elem_size_bytes%256 assert (stride is 256B)."""
    gp = nc.gpsimd
    _in_ap = gp.lower_ap_dma(in_ap, for_custom_bir_dma=True)
    _idxs_ap = gp.lower_ap(idxs_ap)
    _out_ap = gp.lower_ap(out_ap)
    stride_bytes = in_ap.ap[0][0] * mybir.dt.size(in_ap.dtype)
    assert stride_bytes % 256 == 0
    return gp.add_instruction(
        mybir.InstDMAGatherAnt(
            name=nc.get_next_instruction_name(),
            ins=[*_in_ap, _idxs_ap, gp.lower_val_access(gp.to_reg(num_idxs))],
            outs=[_out_ap],
            transpose=False,
            num_idxs=num_idxs,
            elem_size=elem_size,
            stride_bytes_256=stride_bytes // 256,
            gen_mode=0,
            single_packet=single_packet,
            queue_num=queue_num,
            sbuf_tokens_per_rank=0,
            sbuf_free_dim_per_rank=0,
            sbuf_free_dim_pad_per_rank=0,
            sbuf_byte_offset=0,
        )
    )


def build(g_sched, reps: int = 1, mode: str = "full", qpat=None):
    """Build + compile the per-core Bass module.

    g_sched: [NBLK][NQ] gather slot counts (>=1 each).
    reps > 1 wraps the block loop in tc.For_i for slope timing.
    mode: "full" | "gather" (skip reduce/scale/out) | "nored" (skip reduce).
    """
    g_sched = [list(r) for r in g_sched]
    assert len(g_sched) == NBLK and all(len(r) == NQ for r in g_sched)
    gtot = [sum(r) for r in g_sched]
    g_max = max(gtot)
    # idx16 column layout: per (block, pass) a run of G*P/16 int16 columns
    wcols = [[g * P // 16 for g in r] for r in g_sched]
    WC = sum(sum(r) for r in wcols)

    nc = bacc.Bacc("TRN2", target_bir_lowering=False, debug=False,
                   num_swdge_queues=NQ)
    table = nc.dram_tensor("table", [DEVROWS, P], mybir.dt.float16,
                           kind="ExternalInput")
    idx = nc.dram_tensor("idx", [P, WC], mybir.dt.int16, kind="ExternalInput")
    inv_len = nc.dram_tensor("inv_len", [P, NBLK], mybir.dt.float32,
                             kind="ExternalInput")
    out = nc.dram_tensor("out", [NBLK, P, D], mybir.dt.float32,
                         kind="ExternalOutput")

    with tile.TileContext(nc) as tc:
        with (
            tc.tile_pool(name="const", bufs=1) as cpool,
            tc.tile_pool(name="res", bufs=4) as rpool,
        ):
            idx_sb = cpool.tile([P, WC], mybir.dt.int16)
            nc.sync.dma_start(idx_sb[:], idx.ap())
            invl_sb = cpool.tile([P, NBLK], mybir.dt.float32)
            nc.sync.dma_start(invl_sb[:], inv_len.ap())

            # dedicated per-block gather tiles (no rotation): rep r+1's
            # gathers for block b only wait on rep r's reduce of the same
            # block, giving better cross-rep overlap than a rotating pool.
            gtiles = [
                cpool.tile([P, gtot[b], D], mybir.dt.float16, name=f"gt{b}")
                for b in range(NBLK)
            ]

            # one gather per (block, pass).  Queue pattern alternates between
            # blocks so the big outer passes split evenly across queues, while
            # staying periodic in issue order (Tile's DMASW lanes lock to the
            # queue of their first user, so the 8-long pattern must repeat).
            QPAT = qpat or [[0, 1, 2, 3], [2, 3, 0, 1]]

            def body():
                col = 0
                for b in range(NBLK):
                    g = gtiles[b]
                    off = 0
                    for q in range(NQ):
                        gq = g_sched[b][q]
                        win = table.ap()[BASES[q] : BASES[q] + WIN, :D]
                        _manual_dma_gather(
                            nc,
                            g[:, off : off + gq, :],
                            win,
                            idx_sb[:, col : col + wcols[b][q]],
                            gq * P,
                            D,
                            queue_num=QPAT[b % 2][q],
                            single_packet=(gq <= 8),
                        )
                        off += gq
                        col += wcols[b][q]
                    if mode == "gather":
                        continue
                    red = rpool.tile([P, D], mybir.dt.float32, tag="red")
                    if mode == "nored":
                        nc.vector.tensor_copy(red[:], g[:, 0, :])
                    else:
                        nc.vector.tensor_reduce(
                            out=red[:],
                            in_=g[:, : gtot[b], :].rearrange("p l d -> p d l"),
                            axis=mybir.AxisListType.X,
                            op=mybir.AluOpType.add,
                        )
                    o = rpool.tile([P, D], mybir.dt.float32, tag="o")
                    nc.scalar.activation(
                        o[:],
                        red[:],
                        mybir.ActivationFunctionType.Copy,
                        scale=invl_sb[:, b : b + 1],
                    )
                    nc.sync.dma_start(out.ap()[b], o[:])

            if reps == 1:
                body()
            else:
                with tc.For_i(0, reps, 1):
                    body()

    nc.compile()
    return nc


def _dev_table(table):
    """fp16 device table [DEVROWS, 128]: zero sentinel at each window base,
    original row r at device position devpos[r]."""
    t16 = np.asarray(table, dtype=np.float32).astype(np.float16)
    dev = np.zeros((DEVROWS, P), np.float16)
    devpos = np.empty(V, np.int64)
    src = 0
    for pos in range(DEVROWS):
        if pos in BASES:
            continue  # zero sentinel
        dev[pos, :D] = t16[src]
        devpos[src] = pos
        src += 1
    assert src == V
    return dev, devpos


def _balance_passes(devrows_sample):
    """Assign each device-row index to a feasible pass, equalizing per-pass
    counts. Returns list of NQ lists of window-relative indices."""
    groups = [[] for _ in range(NQ)]
    flex = []
    for d in devrows_sample:
        feas = [q for q in range(NQ) if BASES[q] <= d < BASES[q] + WIN]
        if len(feas) == 1:
            groups[feas[0]].append(d - BASES[feas[0]])
        else:
            flex.append((d, feas))
    for d, feas in flex:
        q = min(feas, key=lambda q: len(groups[q]))
        groups[q].append(d - BASES[q])
    return groups


def preprocess(table, indices, lengths):
    """Host prep. Returns (in_maps, g_sched, order) where order[r] is the
    original sample id at global dealt rank r."""
    dev, devpos = _dev_table(table)

    idx_np = np.asarray(indices, dtype=np.int64)  # [B, L]
    lens = np.asarray(lengths).astype(np.int64)  # [B]
    inv_len = (1.0 / np.maximum(lens, 1)).astype(np.float32)

    # per-sample pass groups (window-relative indices)
    sample_groups = []
    cnt = np.zeros((B, NQ), np.int64)
    for s in range(B):
        drows = devpos[idx_np[s, : lens[s]]]
        sample_groups.append(_balance_passes(drows))
        cnt[s] = [len(g) for g in sample_groups[s]]

    # greedy deal: assign samples to the 16 global blocks (1024 each) to
    # minimize the per-block per-pass maxima; rank r -> (block r//1024,
    # core (r%1024)//128, partition r%128)
    key = cnt.max(1) * 64 + lens
    pool = np.argsort(-key, kind="stable")
    gmax = np.zeros((NBLK, NQ), np.int64)
    fill = np.zeros(NBLK, np.int64)
    assign = np.empty(B, np.int64)
    for s in pool:
        best, bc = -1, None
        for b in range(NBLK):
            if fill[b] >= 1024:
                continue
            cost = np.maximum(gmax[b], cnt[s]).sum() - gmax[b].sum()
            if bc is None or cost < bc:
                best, bc = b, cost
        assign[s] = best
        gmax[best] = np.maximum(gmax[best], cnt[s])
        fill[best] += 1
    order = np.concatenate([pool[assign[pool] == b] for b in range(NBLK)])

    # within each block, concentrate the biggest profiles in the lowest
    # core so the other cores' trailing slot columns can be -1-trimmed
    cmax = cnt.max(1)
    for b in range(NBLK):
        rb = order[b * 1024 : (b + 1) * 1024]
        order[b * 1024 : (b + 1) * 1024] = rb[np.argsort(-cmax[rb], kind="stable")]

    g_sched = [[int(x) for x in np.maximum(gmax[b], 1)] for b in range(NBLK)]

    wcols = [[g * P // 16 for g in r] for r in g_sched]
    WC = sum(sum(r) for r in wcols)

    in_maps = []
    for c in range(NCORES):
        idx16 = np.zeros((P, WC), np.int16)
        invl_dev = np.empty((P, NBLK), np.float32)
        col = 0
        for b in range(NBLK):
            ranks = order[b * 1024 + c * P : b * 1024 + (c + 1) * P]
            invl_dev[:, b] = inv_len[ranks]
            for q in range(NQ):
                gq = g_sched[b][q]
                blk = np.zeros((P, gq), np.int16)  # sentinel rel idx 0
                m = 1  # per-core max used slots this (block, pass)
                for p, s in enumerate(ranks):
                    grp = sample_groups[s][q]
                    blk[p, : len(grp)] = grp
                    m = max(m, len(grp))
                # stream order i = c*128 + p -> wrap int16 [16, nidx/16] x8
                flat = blk.T.ravel()  # [gq*128]
                w = np.tile(flat.reshape(gq * P // 16, 16).T, (8, 1))
                # columns beyond this core's max are all-pad: mark -1 so the
                # ucode's trailing-negative trim skips their descriptors
                if TRIM:
                    w[:, m * 8 :] = -1
                nw = wcols[b][q]
                idx16[:, col : col + nw] = w
                col += nw
        in_maps.append(
            {
                "table": dev,
                "idx": np.ascontiguousarray(idx16),
                "inv_len": np.ascontiguousarray(invl_dev),
            }
        )
    return in_maps, g_sched, order


def kernel(table, indices, lengths):
    in_maps, g_sched, order = preprocess(table, indices, lengths)
    key = tuple(tuple(r) for r in g_sched)
    nc = _CACHE.get(key)
    if nc is None:
        nc = _CACHE[key] = build(g_sched)
    res = bass_utils.run_bass_kernel_spmd(nc, in_maps, core_ids=list(range(NCORES)))
    full = np.empty((B, D), np.float32)
    for b in range(NBLK):
        for c in range(NCORES):
            ranks = order[b * 1024 + c * P : b * 1024 + (c + 1) * P]
            full[ranks] = res.results[c]["out"][b]
    return full


# revision 25
# speedup vs baseline: 1.1819x; 1.1819x over previous
"""EmbeddingBag-mean (padded ragged gather + masked mean) on 8 Trainium2 cores.

Strategy (data-parallel over batch):
  - Each of the 8 cores owns B/8 = 2048 samples; the embedding table is
    replicated to every core's HBM as fp16, rows padded to 128 elements
    (256 B stride) so the MoE `dma_gather` ucode (stride in 256 B units,
    int16 indices) can gather single 128 B rows.
  - int16 indices only reach 32768 rows, so the gather runs as 4 passes
    over overlapping 32768-row windows (bases ~22412 apart).  A zero
    sentinel row sits at each window base (relative index 0).  Because the
    windows overlap, each sample distributes its indices among feasible
    passes to equalize its per-pass counts, which keeps the per-block
    per-pass slot maxima near len/4 (instead of len/4 + 3 sigma).
  - Samples are globally length-sorted and dealt to (block, core,
    partition) so each block of 128 partitions holds samples of nearly
    equal length; block b needs G[b][q] gather slots for pass q (max over
    the 8 cores' blocks; one SPMD module).  Pad slots -> sentinel row.
  - Device kernel (per core), per block of 128 samples:
      1. 4x dma_gather (one per pass, 4 SWDGE queues round-robin):
         g[p, off_q + j, :] = window_q[idx16[...], :64]   (128 B descs)
      2. one DVE tensor_reduce over all slot columns (fp16 in, fp32 out)
      3. ACT Copy-with-scale by 1/max(len,1) (per-partition scalar)
      4. DMA the [128, 64] fp32 block out
  - Host un-permutes the global deal and returns [B, 64] fp32.
"""

import numpy as np

try:
    import concourse.bacc as bacc
except ImportError:  # harness containers keep the repo at /opt/trn_rl_repo
    import sys

    sys.path.insert(0, "/opt/trn_rl_repo")
    import concourse.bacc as bacc

import concourse.bass as bass
import concourse.mybir as mybir
import concourse.tile as tile
from concourse import bass_utils

B, L, V, D = 16384, 50, 100000, 64
NCORES = 8
P = 128
BC = B // NCORES  # 2048 samples per core
NBLK = BC // P  # 16 blocks of 128 samples
NQ = 4  # gather passes (overlapping windows)
WIN = 32768  # int16-reachable rows per pass
DEVROWS = V + NQ  # table + one zero sentinel per window

# window bases in device-table row space (sentinel zero row at each base)
_SPACING = -(-(DEVROWS - WIN) // (NQ - 1))  # ceil
BASES = [q * _SPACING for q in range(NQ)]
assert BASES[-1] + WIN >= DEVROWS

_CACHE: dict = {}


def _manual_dma_gather(nc, out_ap, in_ap, idxs_ap, num_idxs, elem_size,
                       queue_num, single_packet):
    """dma_gather without the elem_size_bytes%256 assert (stride is 256B)."""
    gp = nc.gpsimd
    _in_ap = gp.lower_ap_dma(in_ap, for_custom_bir_dma=True)
    _idxs_ap = gp.lower_ap(idxs_ap)
    _out_ap = gp.lower_ap(out_ap)
    stride_bytes = in_ap.ap[0][0] * mybir.dt.size(in_ap.dtype)
    assert stride_bytes % 256 == 0
    return gp.add_instruction(
        mybir.InstDMAGatherAnt(
            name=nc.get_next_instruction_name(),
            ins=[*_in_ap, _idxs_ap, gp.lower_val_access(gp.to_reg(num_idxs))],
            outs=[_out_ap],
            transpose=False,
            num_idxs=num_idxs,
            elem_size=elem_size,
            stride_bytes_256=stride_bytes // 256,
            gen_mode=0,
            single_packet=single_packet,
            queue_num=queue_num,
            sbuf_tokens_per_rank=0,
            sbuf_free_dim_per_rank=0,
            sbuf_free_dim_pad_per_rank=0,
            sbuf_byte_offset=0,
        )
    )


def build(g_sched, reps: int = 1, mode: str = "full", qpat=None):
    """Build + compile the per-core Bass module.

    g_sched: [NBLK][NQ] gather slot counts (>=1 each).
    reps > 1 wraps the block loop in tc.For_i for slope timing.
    mode: "full" | "gather" (skip reduce/scale/out) | "nored" (skip reduce).
    """
    g_sched = [list(r) for r in g_sched]
    assert len(g_sched) == NBLK and all(len(r) == NQ for r in g_sched)
    gtot = [sum(r) for r in g_sched]
    g_max = max(gtot)
    # idx16 column layout: per (block, pass) a run of G*P/16 int16 columns
    wcols = [[g * P // 16 for g in r] for r in g_sched]
    WC = sum(sum(r) for r in wcols)

    nc = bacc.Bacc("TRN2", target_bir_lowering=False, debug=False,
                   num_swdge_queues=NQ)
    table = nc.dram_tensor("table", [DEVROWS, P], mybir.dt.float16,
                           kind="ExternalInput")
    idx = nc.dram_tensor("idx", [P, WC], mybir.dt.int16, kind="ExternalInput")
    inv_len = nc.dram_tensor("inv_len", [P, NBLK], mybir.dt.float32,
                             kind="ExternalInput")
    out = nc.dram_tensor("out", [NBLK, P, D], mybir.dt.float32,
                         kind="ExternalOutput")

    with tile.TileContext(nc) as tc:
        with (
            tc.tile_pool(name="const", bufs=1) as cpool,
            tc.tile_pool(name="gather", bufs=4) as gpool,
            tc.tile_pool(name="res", bufs=4) as rpool,
        ):
            idx_sb = cpool.tile([P, WC], mybir.dt.int16)
            nc.sync.dma_start(idx_sb[:], idx.ap())
            invl_sb = cpool.tile([P, NBLK], mybir.dt.float32)
            nc.sync.dma_start(invl_sb[:], inv_len.ap())

            # one gather per (block, pass).  Queue pattern alternates between
            # blocks so the big outer passes split evenly across queues, while
            # staying periodic in issue order (Tile's DMASW lanes lock to the
            # queue of their first user, so the 8-long pattern must repeat).
            QPAT = qpat or [[0, 1, 2, 3], [2, 3, 0, 1]]

            def body():
                col = 0
                for b in range(NBLK):
                    g = gpool.tile([P, g_max, D], mybir.dt.float16, tag="g")
                    off = 0
                    for q in range(NQ):
                        gq = g_sched[b][q]
                        win = table.ap()[BASES[q] : BASES[q] + WIN, :D]
                        _manual_dma_gather(
                            nc,
                            g[:, off : off + gq, :],
                            win,
                            idx_sb[:, col : col + wcols[b][q]],
                            gq * P,
                            D,
                            queue_num=QPAT[b % 2][q],
                            single_packet=(gq <= 8),
                        )
                        off += gq
                        col += wcols[b][q]
                    if mode == "gather":
                        continue
                    red = rpool.tile([P, D], mybir.dt.float32, tag="red")
                    if mode == "nored":
                        nc.vector.tensor_copy(red[:], g[:, 0, :])
                    else:
                        nc.vector.tensor_reduce(
                            out=red[:],
                            in_=g[:, : gtot[b], :].rearrange("p l d -> p d l"),
                            axis=mybir.AxisListType.X,
                            op=mybir.AluOpType.add,
                        )
                    o = rpool.tile([P, D], mybir.dt.float32, tag="o")
                    nc.scalar.activation(
                        o[:],
                        red[:],
                        mybir.ActivationFunctionType.Copy,
                        scale=invl_sb[:, b : b + 1],
                    )
                    nc.sync.dma_start(out.ap()[b], o[:])

            if reps == 1:
                body()
            else:
                with tc.For_i(0, reps, 1):
                    body()

    nc.compile()
    return nc


def _dev_table(table):
    """fp16 device table [DEVROWS, 128]: zero sentinel at each window base,
    original row r at device position devpos[r]."""
    t16 = np.asarray(table, dtype=np.float32).astype(np.float16)
    dev = np.zeros((DEVROWS, P), np.float16)
    devpos = np.empty(V, np.int64)
    src = 0
    for pos in range(DEVROWS):
        if pos in BASES:
            continue  # zero sentinel
        dev[pos, :D] = t16[src]
        devpos[src] = pos
        src += 1
    assert src == V
    return dev, devpos


def _balance_passes(devrows_sample):
    """Assign each device-row index to a feasible pass, equalizing per-pass
    counts. Returns list of NQ lists of window-relative indices."""
    groups = [[] for _ in range(NQ)]
    flex = []
    for d in devrows_sample:
        feas = [q for q in range(NQ) if BASES[q] <= d < BASES[q] + WIN]
        if len(feas) == 1:
            groups[feas[0]].append(d - BASES[feas[0]])
        else:
            flex.append((d, feas))
    for d, feas in flex:
        q = min(feas, key=lambda q: len(groups[q]))
        groups[q].append(d - BASES[q])
    return groups


def preprocess(table, indices, lengths):
    """Host prep. Returns (in_maps, g_sched, order) where order[r] is the
    original sample id at global dealt rank r."""
    dev, devpos = _dev_table(table)

    idx_np = np.asarray(indices, dtype=np.int64)  # [B, L]
    lens = np.asarray(lengths).astype(np.int64)  # [B]
    inv_len = (1.0 / np.maximum(lens, 1)).astype(np.float32)

    # per-sample pass groups (window-relative indices)
    sample_groups = []
    cnt = np.zeros((B, NQ), np.int64)
    for s in range(B):
        drows = devpos[idx_np[s, : lens[s]]]
        sample_groups.append(_balance_passes(drows))
        cnt[s] = [len(g) for g in sample_groups[s]]

    # greedy deal: assign samples to the 16 global blocks (1024 each) to
    # minimize the per-block per-pass maxima; rank r -> (block r//1024,
    # core (r%1024)//128, partition r%128)
    key = cnt.max(1) * 64 + lens
    pool = np.argsort(-key, kind="stable")
    gmax = np.zeros((NBLK, NQ), np.int64)
    fill = np.zeros(NBLK, np.int64)
    assign = np.empty(B, np.int64)
    for s in pool:
        best, bc = -1, None
        for b in range(NBLK):
            if fill[b] >= 1024:
                continue
            cost = np.maximum(gmax[b], cnt[s]).sum() - gmax[b].sum()
            if bc is None or cost < bc:
                best, bc = b, cost
        assign[s] = best
        gmax[best] = np.maximum(gmax[best], cnt[s])
        fill[best] += 1
    order = np.concatenate([pool[assign[pool] == b] for b in range(NBLK)])

    g_sched = [[int(x) for x in np.maximum(gmax[b], 1)] for b in range(NBLK)]

    wcols = [[g * P // 16 for g in r] for r in g_sched]
    WC = sum(sum(r) for r in wcols)

    in_maps = []
    for c in range(NCORES):
        idx16 = np.zeros((P, WC), np.int16)
        invl_dev = np.empty((P, NBLK), np.float32)
        col = 0
        for b in range(NBLK):
            ranks = order[b * 1024 + c * P : b * 1024 + (c + 1) * P]
            invl_dev[:, b] = inv_len[ranks]
            for q in range(NQ):
                gq = g_sched[b][q]
                blk = np.zeros((P, gq), np.int16)  # sentinel rel idx 0
                for p, s in enumerate(ranks):
                    grp = sample_groups[s][q]
                    blk[p, : len(grp)] = grp
                # stream order i = c*128 + p -> wrap int16 [16, nidx/16] x8
                flat = blk.T.ravel()  # [gq*128]
                w = flat.reshape(gq * P // 16, 16).T  # [16, cols]
                nw = wcols[b][q]
                idx16[:, col : col + nw] = np.tile(w, (8, 1))
                col += nw
        in_maps.append(
            {
                "table": dev,
                "idx": np.ascontiguousarray(idx16),
                "inv_len": np.ascontiguousarray(invl_dev),
            }
        )
    return in_maps, g_sched, order


def kernel(table, indices, lengths):
    in_maps, g_sched, order = preprocess(table, indices, lengths)
    key = tuple(tuple(r) for r in g_sched)
    nc = _CACHE.get(key)
    if nc is None:
        nc = _CACHE[key] = build(g_sched)
    res = bass_utils.run_bass_kernel_spmd(nc, in_maps, core_ids=list(range(NCORES)))
    full = np.empty((B, D), np.float32)
    for b in range(NBLK):
        for c in range(NCORES):
            ranks = order[b * 1024 + c * P : b * 1024 + (c + 1) * P]
            full[ranks] = res.results[c]["out"][b]
    return full
